# revision 11
# baseline (speedup 1.0000x reference)
"""Trainium2 Bass kernel for nn_FGNet (gnn_message_passing) — v3 (fp16 + PE tiling).

Strategy
--------
Edges sorted by type id, uniform 256-edge blocks (one id per block, padded;
2 segments x 128 edges), processed in PAIRS (blocks 2q, 2q+1).  All device
tensors fp16 (PSUM accumulation stays f32); rel err ~1e-3 vs the 2e-2 gate.

Per pair q, with block a on SBUF partitions 0:64 and block b on 64:128:

    mm1 (K=64, row-tiled): T(0,0) computes W_a.T @ feats_a while T(64,0)
        computes W_b.T @ feats_b concurrently; N=384 halves (PSUM-bank aligned)
    relu+bias (ACT): psum f32 -> t fp16, one 3D-AP instruction per block
    products (DVE): p_i = t_j * t_k, fp16 at 2 elem/cycle, 3 per block
    mm2 (M=64, col-tiled): T(0,0) writes msg_a to psum partitions 0:64 while
        T(0,64) writes msg_b to 64:128 -> natural [128,768] pair layout
    copy (DVE): psum f32 -> m fp16 [128,768], one instruction per pair
    (second bias b2 is linear in the segment-sum -> folded to the host)

HW constraints baked in (validated on this stack by mb2.py):
  - matmul output must lie within one 2KB PSUM bank (N<=512 f32, no crossing)
  - DMA cannot touch PSUM; GPSIMD cannot touch PSUM
  - PE tiling via tile_position works for fp16 (row tiles need lhsT/rhs on
    the matching SBUF partition half; col tiles write psum partition halves)
  - f32r needs K=128 (v2 legacy); fp16 K=64 is fine

Packed inputs per pair (host-side numpy):
    fk  [128, 768] fp16   partition 64c+l, col = seg*384 + i*128 + e
    wk  [128, 128] fp16   rows 0:64 = W_a, rows 64:128 = W_b
    hot [128, 384] fp16   col 192c + 64i + l = ho_params[i, id_c, r, l]
    bia [128, B]   f32    column 2q+c = bias_p[id_c]
Output msgs[q] [128, 768] fp16: partition 64c+l, col = i*256 + seg*128 + e.

Host side: id computation, sort, feature gather, packing, unpermute, b2 bias
add and the final segment-sum into node_msg.
"""

import numpy as np

_BLK = 256          # edge slots per block (2 segments x 128)
_SEG = 128
_NCORES = 8

_prog_cache = {}


def _build_program(P):
    """Build the SPMD device program for P block-pairs per core."""
    import concourse.mybir as mybir
    import concourse.tile as tile
    from concourse import bacc
    from concourse.bass import broadcast_tensor_aps

    F32 = mybir.dt.float32
    F16 = mybir.dt.float16
    Relu = mybir.ActivationFunctionType.Relu
    Copy = mybir.ActivationFunctionType.Copy

    B = 2 * P
    nc = bacc.Bacc()
    pkin = nc.declare_dram_parameter("pkin", [P, 128, 1280], F16, isOutput=False)
    bia = nc.declare_dram_parameter("bia", [128, B], F32, isOutput=False)
    msgs = nc.declare_dram_parameter("msgs", [P, 128, 768], F16, isOutput=True)

    with tile.TileContext(nc) as tc:
        with (
            tc.tile_pool(name="const", bufs=1) as const,
            tc.tile_pool(name="work", bufs=6) as work,
            tc.tile_pool(name="psA", bufs=1, space="PSUM") as psA,
            tc.tile_pool(name="psB", bufs=2, space="PSUM") as psB,
        ):
            bt = const.tile([128, B], F32, name="bt")
            nc.sync.dma_start(out=bt[:], in_=bia[:])

            for q in range(P):
                # one merged input DMA per pair: feats | W | ho
                pkt = work.tile([128, 1280], F16, name="pkt", tag="pkt")
                nc.sync.dma_start(out=pkt[:], in_=pkin[q])
                fkt = pkt[:, 0:768]
                wkt = pkt[:, 768:896]

                # mm1: row-tiled K=64 pair, N=384 bank-aligned halves
                ps_a = psA.tile([128, 2, 512], F32, name="ps_a", tag="ps_a")
                ps_b = psA.tile([128, 2, 512], F32, name="ps_b", tag="ps_b")
                for j in range(2):
                    nc.tensor.matmul(
                        out=ps_a[:, j, 0:384], lhsT=wkt[0:64, :],
                        rhs=fkt[0:64, 384 * j:384 * (j + 1)],
                        start=True, stop=True, tile_position=(0, 0))
                    nc.tensor.matmul(
                        out=ps_b[:, j, 0:384], lhsT=wkt[64:128, :],
                        rhs=fkt[64:128, 384 * j:384 * (j + 1)],
                        start=True, stop=True, tile_position=(64, 0))

                # t cols are pos-major with storage order pos0=t_1, pos1=t_0,
                # pos2=t_2 so the first two products share one strided op:
                #   p[0:2] = t[pos 0:2] * t[pos2 broadcast] = (t1*t2, t0*t2)
                #   p[2]   = t[pos1] * t[pos0]              = t0*t1
                ta = work.tile([128, 3, 256], F16, name="ta", tag="ta")
                tb = work.tile([128, 3, 256], F16, name="tb", tag="tb")
                nc.scalar.activation(
                    out=ta[:].rearrange("r p e -> r (p e)").rearrange(
                        "r (j x) -> r j x", j=2),
                    in_=ps_a[:, :, 0:384], func=Relu,
                    bias=bt[:, 2 * q:2 * q + 1], scale=1.0)
                nc.vector.tensor_scalar(
                    out=tb[:].rearrange("r p e -> r (p e)").rearrange(
                        "r (j x) -> r j x", j=2),
                    in0=ps_b[:, :, 0:384],
                    scalar1=bt[:, 2 * q + 1:2 * q + 2], scalar2=0.0,
                    op0=mybir.AluOpType.add, op1=mybir.AluOpType.max)

                pa = work.tile([128, 3, 256], F16, name="pa", tag="pa")
                pb = work.tile([128, 3, 256], F16, name="pb", tag="pb")
                for t_, p_ in ((ta, pa), (tb, pb)):
                    i0, i1 = broadcast_tensor_aps(t_[:, 0:2, :], t_[:, 2:3, :])
                    nc.vector.tensor_mul(out=p_[:, 0:2, :], in0=i0, in1=i1)
                    nc.gpsimd.tensor_mul(out=p_[:, 2, :], in0=t_[:, 1, :],
                                         in1=t_[:, 0, :])

                # mm2: col-tiled M=64 pairs into psum partition halves
                ps2 = psB.tile([128, 3, 256], F32, name="ps2", tag="ps2")
                for i in range(3):
                    nc.tensor.matmul(
                        out=ps2[0:64, i, :],
                        lhsT=pkt[:, 896 + 64 * i:896 + 64 * (i + 1)],
                        rhs=pa[:, i, :],
                        start=True, stop=True, tile_position=(0, 0))
                    nc.tensor.matmul(
                        out=ps2[64:128, i, :],
                        lhsT=pkt[:, 1088 + 64 * i:1088 + 64 * (i + 1)],
                        rhs=pb[:, i, :],
                        start=True, stop=True, tile_position=(0, 64))

                # psum evacuation on scalar, one instruction
                m = work.tile([128, 768], F16, name="m", tag="m")
                nc.scalar.activation(
                    out=m[:], in_=ps2[:].rearrange("l i c -> l (i c)"),
                    func=Copy, bias=0.0, scale=1.0)
                nc.scalar.dma_start(out=msgs[q], in_=m[:])
    nc.finalize()
    return nc


def _get_program(P):
    if P not in _prog_cache:
        _prog_cache[P] = _build_program(P)
    return _prog_cache[P]


def _prepare(x, nodes, fact, params, bias_p, ho_params, ho_bias):
    """Host-side: sort by id, build per-pair packed fp16 arrays."""
    N, L = nodes.shape
    E = fact.shape[0]
    R = params.shape[2]
    NP = params.shape[0]           # 169
    MA = int(round(NP ** 0.5))     # 13

    ids = (x[fact[:, 0], 1] * MA + x[fact[:, 0], 2]).astype(np.int64)   # [E]
    perm = np.argsort(ids, kind="stable")
    ids_s = ids[perm]
    fact_s = fact[perm].astype(np.int64)                                 # [E,3]

    counts = np.bincount(ids_s, minlength=NP)                            # [NP]
    nblk = (counts + _BLK - 1) // _BLK                                   # [NP]
    blk_ids = np.repeat(np.arange(NP), nblk)                             # [NB]
    NB = int(blk_ids.shape[0])
    B = (NB + _NCORES - 1) // _NCORES
    if B % 2:
        B += 1
    NB8 = B * _NCORES
    blk_ids = np.concatenate([blk_ids, np.zeros(NB8 - NB, np.int64)])

    # slot -> sorted-edge-position map (-1 = padding)
    padded = nblk * _BLK
    pad_off = np.concatenate([[0], np.cumsum(padded)])
    off = np.concatenate([[0], np.cumsum(counts)])
    total = int(pad_off[-1])
    t_of = np.repeat(np.arange(NP), padded)
    jloc = np.arange(total) - pad_off[t_of]
    src = np.where(jloc < counts[t_of], off[t_of] + jloc, -1)
    src = np.concatenate([src, np.full(NB8 * _BLK - total, -1, np.int64)])
    valid = src >= 0

    # gather features per slot
    nf = nodes[fact_s].astype(np.float16)                                # [E,3,L]
    featp = np.zeros((NB8 * _BLK, 3, L), np.float16)
    featp[valid] = nf[src[valid]]

    NPAIR = NB8 // 2
    # pkin: cols 0:768 feats, 768:896 W, 896:1280 ho
    pkin = np.zeros((NPAIR, 128, 1280), np.float16)
    # feats: [q, 64c+l, pos*256 + seg*128 + e], storage order pos = (1, 0, 2)
    pkin[:, :, 0:768] = (
        featp[:, [1, 0, 2], :]                      # slot pos l
        .reshape(NPAIR, 2, 2, _SEG, 3, L)           # q c seg e pos l
        .transpose(0, 1, 5, 4, 2, 3)                # q c l pos seg e
        .reshape(NPAIR, 128, 768))
    # W: rows 0:64 = W_a, 64:128 = W_b
    pkin[:, :, 768:896] = (
        params[blk_ids].astype(np.float16)          # [NB8, L, R]
        .reshape(NPAIR, 2 * L, R))
    # ho: [q, r, 192c + 64i + l]
    pkin[:, :, 896:1280] = (
        ho_params[:, blk_ids].astype(np.float16)    # [3, NB8, R, L]
        .transpose(1, 2, 0, 3)                      # NB8 r i l
        .reshape(NPAIR, 2, R, 3 * L)                # q c r (i l)
        .transpose(0, 2, 1, 3)                      # q r c (i l)
        .reshape(NPAIR, R, 384))

    biasT = bias_p[blk_ids, 0].astype(np.float32)                        # [NB8,R]
    biasT = biasT.reshape(_NCORES, B, R).transpose(0, 2, 1)              # [8,R,B]

    return dict(pkin=pkin, biasT=np.ascontiguousarray(biasT),
                B=B, NB8=NB8, P=B // 2,
                src=src, valid=valid, fact_s=fact_s, ids_s=ids_s,
                N=N, E=E, L=L)


def _postprocess(msgs_all, prep, ho_bias):
    """Decode per-slot messages, add host-side b2, segment-sum into node_msg."""
    NB8, N, E, L = prep["NB8"], prep["N"], prep["E"], prep["L"]
    src, valid, fact_s, ids_s = prep["src"], prep["valid"], prep["fact_s"], prep["ids_s"]
    NPAIR = NB8 // 2
    # msgs_all [NPAIR, 128, 768]: partition 64c+l, col = i*256 + seg*128 + e
    slots = (
        msgs_all.astype(np.float32)
        .reshape(NPAIR, 2, 64, 3, 2, _SEG)          # q c l i seg e
        .transpose(0, 1, 4, 5, 3, 2)                # q c seg e i l
        .reshape(NB8 * _BLK, 3, 64)
    )
    msg_e = np.empty((E, 3, L), np.float32)
    msg_e[src[valid]] = slots[valid]

    # fold in the second bias (linear in the segment-sum)
    msg_e += ho_bias[:, ids_s, 0].astype(np.float32).transpose(1, 0, 2)  # [E,3,L]

    idx_all = fact_s.T.reshape(-1)                                       # [3E]
    val_all = msg_e.transpose(1, 0, 2).reshape(-1, L)                    # [3E,L]
    order = np.argsort(idx_all, kind="stable")
    idx_sorted = idx_all[order]
    val_sorted = val_all[order]
    uniq, starts = np.unique(idx_sorted, return_index=True)
    sums = np.add.reduceat(val_sorted, starts, axis=0)
    out = np.zeros((N, L), np.float32)
    out[uniq] = sums
    return out


def _run_device(prep, trace=False, trace_kwargs=None):
    from concourse.bass_utils import run_bass_kernel_spmd

    P = prep["P"]
    nc = _get_program(P)
    in_maps = []
    for c in range(_NCORES):
        in_maps.append({
            "pkin": prep["pkin"][c * P:(c + 1) * P],
            "bia": prep["biasT"][c],
        })
    kwargs = {}
    if trace:
        kwargs["trace"] = True
        if trace_kwargs:
            kwargs.update(trace_kwargs)
    res = run_bass_kernel_spmd(nc, in_maps, list(range(_NCORES)), **kwargs)
    msgs_all = np.concatenate([res.results[c]["msgs"] for c in range(_NCORES)],
                              axis=0)
    return msgs_all, res


def kernel(x, nodes, fact, fact_dim, params, bias_p, ho_params, ho_bias,
           _trace=False, _trace_kwargs=None):
    x = np.asarray(x)
    nodes = np.asarray(nodes, dtype=np.float32)
    fact = np.asarray(fact)
    params = np.asarray(params)
    bias_p = np.asarray(bias_p)
    ho_params = np.asarray(ho_params)
    ho_bias = np.asarray(ho_bias)

    prep = _prepare(x, nodes, fact, params, bias_p, ho_params, ho_bias)
    msgs_all, res = _run_device(prep, trace=_trace, trace_kwargs=_trace_kwargs)
    out = _postprocess(msgs_all, prep, ho_bias)
    kernel.last_results = res
    return out


# revision 15
# speedup vs baseline: 1.0703x; 1.0703x over previous
"""Trainium2 Bass kernel for nn_FGNet (gnn_message_passing) — v3 (fp16 + PE tiling).

Strategy
--------
Edges sorted by type id, uniform 256-edge blocks (one id per block, padded;
2 segments x 128 edges), processed in PAIRS (blocks 2q, 2q+1).  All device
tensors fp16 (PSUM accumulation stays f32); rel err ~1e-3 vs the 2e-2 gate.

Per pair q, with block a on SBUF partitions 0:64 and block b on 64:128:

    mm1 (K=64, row-tiled): T(0,0) computes W_a.T @ feats_a while T(64,0)
        computes W_b.T @ feats_b concurrently; N=384 halves (PSUM-bank aligned)
    relu+bias (ACT): psum f32 -> t fp16, one 3D-AP instruction per block
    products (DVE): p_i = t_j * t_k, fp16 at 2 elem/cycle, 3 per block
    mm2 (M=64, col-tiled): T(0,0) writes msg_a to psum partitions 0:64 while
        T(0,64) writes msg_b to 64:128 -> natural [128,768] pair layout
    copy (DVE): psum f32 -> m fp16 [128,768], one instruction per pair
    (second bias b2 is linear in the segment-sum -> folded to the host)

HW constraints baked in (validated on this stack by mb2.py):
  - matmul output must lie within one 2KB PSUM bank (N<=512 f32, no crossing)
  - DMA cannot touch PSUM; GPSIMD cannot touch PSUM
  - PE tiling via tile_position works for fp16 (row tiles need lhsT/rhs on
    the matching SBUF partition half; col tiles write psum partition halves)
  - f32r needs K=128 (v2 legacy); fp16 K=64 is fine

Packed inputs per pair (host-side numpy):
    fk  [128, 768] fp16   partition 64c+l, col = seg*384 + i*128 + e
    wk  [128, 128] fp16   rows 0:64 = W_a, rows 64:128 = W_b
    hot [128, 384] fp16   col 192c + 64i + l = ho_params[i, id_c, r, l]
    bia [128, B]   f32    column 2q+c = bias_p[id_c]
Output msgs[q] [128, 768] fp16: partition 64c+l, col = i*256 + seg*128 + e.

Host side: id computation, sort, feature gather, packing, unpermute, b2 bias
add and the final segment-sum into node_msg.
"""

import numpy as np

_BLK = 256          # edge slots per block (2 segments x 128)
_SEG = 128
_NCORES = 8

_prog_cache = {}


def _build_program(P):
    """Build the SPMD device program for P block-pairs per core."""
    import concourse.mybir as mybir
    import concourse.tile as tile
    from concourse import bacc
    from concourse.bass import broadcast_tensor_aps

    F32 = mybir.dt.float32
    F16 = mybir.dt.float16
    Relu = mybir.ActivationFunctionType.Relu
    Copy = mybir.ActivationFunctionType.Copy

    B = 2 * P
    nc = bacc.Bacc()
    pkin = nc.declare_dram_parameter("pkin", [P, 128, 1280], F16, isOutput=False)
    bia = nc.declare_dram_parameter("bia", [128, B], F32, isOutput=False)
    msgs = nc.declare_dram_parameter("msgs", [P, 128, 768], F16, isOutput=True)

    with tile.TileContext(nc) as tc:
        with (
            tc.tile_pool(name="const", bufs=1) as const,
            tc.tile_pool(name="work", bufs=6) as work,
            tc.tile_pool(name="psA", bufs=1, space="PSUM") as psA,
            tc.tile_pool(name="psB", bufs=2, space="PSUM") as psB,
        ):
            bt = const.tile([128, B], F32, name="bt")
            nc.sync.dma_start(out=bt[:], in_=bia[:])

            def emit_back(prev):
                # software-pipelined tail of pair q-1: mm2 + evac + store
                pkt, pa, pb, q = prev
                ps2 = psB.tile([128, 3, 256], F32, name="ps2", tag="ps2")
                for i in range(3):
                    nc.tensor.matmul(
                        out=ps2[0:64, i, :],
                        lhsT=pkt[:, 896 + 64 * i:896 + 64 * (i + 1)],
                        rhs=pa[:, i, :],
                        start=True, stop=True, tile_position=(0, 0))
                    nc.tensor.matmul(
                        out=ps2[64:128, i, :],
                        lhsT=pkt[:, 1088 + 64 * i:1088 + 64 * (i + 1)],
                        rhs=pb[:, i, :],
                        start=True, stop=True, tile_position=(0, 64))
                m = work.tile([128, 768], F16, name="m", tag="m")
                nc.scalar.activation(
                    out=m[:], in_=ps2[:].rearrange("l i c -> l (i c)"),
                    func=Copy, bias=0.0, scale=1.0)
                nc.scalar.dma_start(out=msgs[q], in_=m[:])

            prev = None
            for q in range(P):
                # one merged input DMA per pair: feats | W | ho
                pkt = work.tile([128, 1280], F16, name="pkt", tag="pkt")
                nc.sync.dma_start(out=pkt[:], in_=pkin[q])
                fkt = pkt[:, 0:768]
                wkt = pkt[:, 768:896]

                # mm1: row-tiled K=64 pair, N=384 bank-aligned halves
                ps_a = psA.tile([128, 2, 512], F32, name="ps_a", tag="ps_a")
                ps_b = psA.tile([128, 2, 512], F32, name="ps_b", tag="ps_b")
                for j in range(2):
                    nc.tensor.matmul(
                        out=ps_a[:, j, 0:384], lhsT=wkt[0:64, :],
                        rhs=fkt[0:64, 384 * j:384 * (j + 1)],
                        start=True, stop=True, tile_position=(0, 0))
                    nc.tensor.matmul(
                        out=ps_b[:, j, 0:384], lhsT=wkt[64:128, :],
                        rhs=fkt[64:128, 384 * j:384 * (j + 1)],
                        start=True, stop=True, tile_position=(64, 0))

                # t cols pos-major, storage order pos0=t_1, pos1=t_0, pos2=t_2
                # relu_a on ACT, relu_b on DVE (parallel engines)
                ta = work.tile([128, 3, 256], F16, name="ta", tag="ta")
                tb = work.tile([128, 3, 256], F16, name="tb", tag="tb")
                nc.scalar.activation(
                    out=ta[:].rearrange("r p e -> r (p e)").rearrange(
                        "r (j x) -> r j x", j=2),
                    in_=ps_a[:, :, 0:384], func=Relu,
                    bias=bt[:, 2 * q:2 * q + 1], scale=1.0)
                nc.vector.tensor_scalar(
                    out=tb[:].rearrange("r p e -> r (p e)").rearrange(
                        "r (j x) -> r j x", j=2),
                    in0=ps_b[:, :, 0:384],
                    scalar1=bt[:, 2 * q + 1:2 * q + 2], scalar2=0.0,
                    op0=mybir.AluOpType.add, op1=mybir.AluOpType.max)

                # products: p_0 = pos0*pos2, p_1 = pos1*pos2 on DVE,
                # p_2 = pos1*pos0 on GPSIMD (latency hidden by the pipeline)
                pa = work.tile([128, 3, 256], F16, name="pa", tag="pa")
                pb = work.tile([128, 3, 256], F16, name="pb", tag="pb")
                for t_, p_ in ((ta, pa), (tb, pb)):
                    nc.vector.tensor_mul(out=p_[:, 0, :], in0=t_[:, 0, :],
                                         in1=t_[:, 2, :])
                    nc.vector.tensor_mul(out=p_[:, 1, :], in0=t_[:, 1, :],
                                         in1=t_[:, 2, :])
                    nc.gpsimd.tensor_mul(out=p_[:, 2, :], in0=t_[:, 1, :],
                                         in1=t_[:, 0, :])

                if prev is not None:
                    emit_back(prev)
                prev = (pkt, pa, pb, q)
            emit_back(prev)
    nc.finalize()
    return nc


def _get_program(P):
    if P not in _prog_cache:
        _prog_cache[P] = _build_program(P)
    return _prog_cache[P]


def _prepare(x, nodes, fact, params, bias_p, ho_params, ho_bias):
    """Host-side: sort by id, build per-pair packed fp16 arrays."""
    N, L = nodes.shape
    E = fact.shape[0]
    R = params.shape[2]
    NP = params.shape[0]           # 169
    MA = int(round(NP ** 0.5))     # 13

    ids = (x[fact[:, 0], 1] * MA + x[fact[:, 0], 2]).astype(np.int64)   # [E]
    perm = np.argsort(ids, kind="stable")
    ids_s = ids[perm]
    fact_s = fact[perm].astype(np.int64)                                 # [E,3]

    counts = np.bincount(ids_s, minlength=NP)                            # [NP]
    nblk = (counts + _BLK - 1) // _BLK                                   # [NP]
    blk_ids = np.repeat(np.arange(NP), nblk)                             # [NB]
    NB = int(blk_ids.shape[0])
    B = (NB + _NCORES - 1) // _NCORES
    if B % 2:
        B += 1
    NB8 = B * _NCORES
    blk_ids = np.concatenate([blk_ids, np.zeros(NB8 - NB, np.int64)])

    # slot -> sorted-edge-position map (-1 = padding)
    padded = nblk * _BLK
    pad_off = np.concatenate([[0], np.cumsum(padded)])
    off = np.concatenate([[0], np.cumsum(counts)])
    total = int(pad_off[-1])
    t_of = np.repeat(np.arange(NP), padded)
    jloc = np.arange(total) - pad_off[t_of]
    src = np.where(jloc < counts[t_of], off[t_of] + jloc, -1)
    src = np.concatenate([src, np.full(NB8 * _BLK - total, -1, np.int64)])
    valid = src >= 0

    # gather features per slot
    nf = nodes[fact_s].astype(np.float16)                                # [E,3,L]
    featp = np.zeros((NB8 * _BLK, 3, L), np.float16)
    featp[valid] = nf[src[valid]]

    NPAIR = NB8 // 2
    # pkin: cols 0:768 feats, 768:896 W, 896:1280 ho
    pkin = np.zeros((NPAIR, 128, 1280), np.float16)
    # feats: [q, 64c+l, pos*256 + seg*128 + e], storage order pos = (1, 0, 2)
    pkin[:, :, 0:768] = (
        featp[:, [1, 0, 2], :]                      # slot pos l
        .reshape(NPAIR, 2, 2, _SEG, 3, L)           # q c seg e pos l
        .transpose(0, 1, 5, 4, 2, 3)                # q c l pos seg e
        .reshape(NPAIR, 128, 768))
    # W: rows 0:64 = W_a, 64:128 = W_b
    pkin[:, :, 768:896] = (
        params[blk_ids].astype(np.float16)          # [NB8, L, R]
        .reshape(NPAIR, 2 * L, R))
    # ho: [q, r, 192c + 64i + l]
    pkin[:, :, 896:1280] = (
        ho_params[:, blk_ids].astype(np.float16)    # [3, NB8, R, L]
        .transpose(1, 2, 0, 3)                      # NB8 r i l
        .reshape(NPAIR, 2, R, 3 * L)                # q c r (i l)
        .transpose(0, 2, 1, 3)                      # q r c (i l)
        .reshape(NPAIR, R, 384))

    biasT = bias_p[blk_ids, 0].astype(np.float32)                        # [NB8,R]
    biasT = biasT.reshape(_NCORES, B, R).transpose(0, 2, 1)              # [8,R,B]

    return dict(pkin=pkin, biasT=np.ascontiguousarray(biasT),
                B=B, NB8=NB8, P=B // 2,
                src=src, valid=valid, fact_s=fact_s, ids_s=ids_s,
                N=N, E=E, L=L)


def _postprocess(msgs_all, prep, ho_bias):
    """Decode per-slot messages, add host-side b2, segment-sum into node_msg."""
    NB8, N, E, L = prep["NB8"], prep["N"], prep["E"], prep["L"]
    src, valid, fact_s, ids_s = prep["src"], prep["valid"], prep["fact_s"], prep["ids_s"]
    NPAIR = NB8 // 2
    # msgs_all [NPAIR, 128, 768]: partition 64c+l, col = i*256 + seg*128 + e
    slots = (
        msgs_all.astype(np.float32)
        .reshape(NPAIR, 2, 64, 3, 2, _SEG)          # q c l i seg e
        .transpose(0, 1, 4, 5, 3, 2)                # q c seg e i l
        .reshape(NB8 * _BLK, 3, 64)
    )
    msg_e = np.empty((E, 3, L), np.float32)
    msg_e[src[valid]] = slots[valid]

    # fold in the second bias (linear in the segment-sum)
    msg_e += ho_bias[:, ids_s, 0].astype(np.float32).transpose(1, 0, 2)  # [E,3,L]

    idx_all = fact_s.T.reshape(-1)                                       # [3E]
    val_all = msg_e.transpose(1, 0, 2).reshape(-1, L)                    # [3E,L]
    order = np.argsort(idx_all, kind="stable")
    idx_sorted = idx_all[order]
    val_sorted = val_all[order]
    uniq, starts = np.unique(idx_sorted, return_index=True)
    sums = np.add.reduceat(val_sorted, starts, axis=0)
    out = np.zeros((N, L), np.float32)
    out[uniq] = sums
    return out


def _run_device(prep, trace=False, trace_kwargs=None):
    from concourse.bass_utils import run_bass_kernel_spmd

    P = prep["P"]
    nc = _get_program(P)
    in_maps = []
    for c in range(_NCORES):
        in_maps.append({
            "pkin": prep["pkin"][c * P:(c + 1) * P],
            "bia": prep["biasT"][c],
        })
    kwargs = {}
    if trace:
        kwargs["trace"] = True
        if trace_kwargs:
            kwargs.update(trace_kwargs)
    res = run_bass_kernel_spmd(nc, in_maps, list(range(_NCORES)), **kwargs)
    msgs_all = np.concatenate([res.results[c]["msgs"] for c in range(_NCORES)],
                              axis=0)
    return msgs_all, res


def kernel(x, nodes, fact, fact_dim, params, bias_p, ho_params, ho_bias,
           _trace=False, _trace_kwargs=None):
    x = np.asarray(x)
    nodes = np.asarray(nodes, dtype=np.float32)
    fact = np.asarray(fact)
    params = np.asarray(params)
    bias_p = np.asarray(bias_p)
    ho_params = np.asarray(ho_params)
    ho_bias = np.asarray(ho_bias)

    prep = _prepare(x, nodes, fact, params, bias_p, ho_params, ho_bias)
    msgs_all, res = _run_device(prep, trace=_trace, trace_kwargs=_trace_kwargs)
    out = _postprocess(msgs_all, prep, ho_bias)
    kernel.last_results = res
    return out


# revision 17
# speedup vs baseline: 1.1338x; 1.0593x over previous
"""Trainium2 Bass kernel for nn_FGNet (gnn_message_passing) — v3 (fp16 + PE tiling).

Strategy
--------
Edges sorted by type id, uniform 256-edge blocks (one id per block, padded;
2 segments x 128 edges), processed in PAIRS (blocks 2q, 2q+1).  All device
tensors fp16 (PSUM accumulation stays f32); rel err ~1e-3 vs the 2e-2 gate.

Per pair q, with block a on SBUF partitions 0:64 and block b on 64:128:

    mm1 (K=64, row-tiled): T(0,0) computes W_a.T @ feats_a while T(64,0)
        computes W_b.T @ feats_b concurrently; N=384 halves (PSUM-bank aligned)
    relu+bias (ACT): psum f32 -> t fp16, one 3D-AP instruction per block
    products (DVE): p_i = t_j * t_k, fp16 at 2 elem/cycle, 3 per block
    mm2 (M=64, col-tiled): T(0,0) writes msg_a to psum partitions 0:64 while
        T(0,64) writes msg_b to 64:128 -> natural [128,768] pair layout
    copy (DVE): psum f32 -> m fp16 [128,768], one instruction per pair
    (second bias b2 is linear in the segment-sum -> folded to the host)

HW constraints baked in (validated on this stack by mb2.py):
  - matmul output must lie within one 2KB PSUM bank (N<=512 f32, no crossing)
  - DMA cannot touch PSUM; GPSIMD cannot touch PSUM
  - PE tiling via tile_position works for fp16 (row tiles need lhsT/rhs on
    the matching SBUF partition half; col tiles write psum partition halves)
  - f32r needs K=128 (v2 legacy); fp16 K=64 is fine

Packed inputs per pair (host-side numpy):
    fk  [128, 768] fp16   partition 64c+l, col = seg*384 + i*128 + e
    wk  [128, 128] fp16   rows 0:64 = W_a, rows 64:128 = W_b
    hot [128, 384] fp16   col 192c + 64i + l = ho_params[i, id_c, r, l]
    bia [128, B]   f32    column 2q+c = bias_p[id_c]
Output msgs[q] [128, 768] fp16: partition 64c+l, col = i*256 + seg*128 + e.

Host side: id computation, sort, feature gather, packing, unpermute, b2 bias
add and the final segment-sum into node_msg.
"""

import numpy as np

_BLK = 256          # edge slots per block (2 segments x 128)
_SEG = 128
_NCORES = 8

_prog_cache = {}


def _build_program(P):
    """Build the SPMD device program for P block-pairs per core."""
    import concourse.mybir as mybir
    import concourse.tile as tile
    from concourse import bacc
    from concourse.bass import broadcast_tensor_aps

    F32 = mybir.dt.float32
    F16 = mybir.dt.float16
    Relu = mybir.ActivationFunctionType.Relu
    Copy = mybir.ActivationFunctionType.Copy

    B = 2 * P
    nc = bacc.Bacc()
    pkin = nc.declare_dram_parameter("pkin", [P, 128, 1280], F16, isOutput=False)
    bia = nc.declare_dram_parameter("bia", [128, B], F32, isOutput=False)
    msgs = nc.declare_dram_parameter("msgs", [P, 128, 768], F16, isOutput=True)

    with tile.TileContext(nc) as tc:
        with (
            tc.tile_pool(name="const", bufs=1) as const,
            tc.tile_pool(name="work", bufs=6) as work,
            tc.tile_pool(name="psA", bufs=1, space="PSUM") as psA,
            tc.tile_pool(name="psB", bufs=2, space="PSUM") as psB,
        ):
            bt = const.tile([128, B], F32, name="bt")
            nc.sync.dma_start(out=bt[:], in_=bia[:])

            def emit_back(prev):
                # software-pipelined tail of pair q-1: mm2 + evac + store
                pkt, pa, pb, q = prev
                ps2 = psB.tile([128, 3, 256], F32, name="ps2", tag="ps2")
                for i in range(3):
                    nc.tensor.matmul(
                        out=ps2[0:64, i, :],
                        lhsT=pkt[:, 896 + 64 * i:896 + 64 * (i + 1)],
                        rhs=pa[:, i, :],
                        start=True, stop=True, tile_position=(0, 0))
                    nc.tensor.matmul(
                        out=ps2[64:128, i, :],
                        lhsT=pkt[:, 1088 + 64 * i:1088 + 64 * (i + 1)],
                        rhs=pb[:, i, :],
                        start=True, stop=True, tile_position=(0, 64))
                ps2f = ps2[:].rearrange("l i c -> l (i c)")
                m = work.tile([128, 768], F16, name="m", tag="m")
                nc.scalar.activation(out=m[:, 0:256], in_=ps2f[:, 0:256],
                                     func=Copy, bias=0.0, scale=1.0)
                nc.vector.tensor_copy(out=m[:, 256:512], in_=ps2f[:, 256:512])
                nc.vector.tensor_copy(out=m[:, 512:768], in_=ps2f[:, 512:768])
                nc.scalar.dma_start(out=msgs[q], in_=m[:])

            prev = None
            for q in range(P):
                # one merged input DMA per pair: feats | W | ho
                pkt = work.tile([128, 1280], F16, name="pkt", tag="pkt")
                nc.sync.dma_start(out=pkt[:], in_=pkin[q])
                fkt = pkt[:, 0:768]
                wkt = pkt[:, 768:896]

                # mm1: row-tiled K=64 pair, N=384 bank-aligned halves
                ps_a = psA.tile([128, 2, 512], F32, name="ps_a", tag="ps_a")
                ps_b = psA.tile([128, 2, 512], F32, name="ps_b", tag="ps_b")
                for j in range(2):
                    nc.tensor.matmul(
                        out=ps_a[:, j, 0:384], lhsT=wkt[0:64, :],
                        rhs=fkt[0:64, 384 * j:384 * (j + 1)],
                        start=True, stop=True, tile_position=(0, 0))
                    nc.tensor.matmul(
                        out=ps_b[:, j, 0:384], lhsT=wkt[64:128, :],
                        rhs=fkt[64:128, 384 * j:384 * (j + 1)],
                        start=True, stop=True, tile_position=(64, 0))

                # t cols pos-major, storage order pos0=t_1, pos1=t_0, pos2=t_2
                # relu_a on ACT, relu_b on DVE (parallel engines)
                ta = work.tile([128, 3, 256], F16, name="ta", tag="ta")
                tb = work.tile([128, 3, 256], F16, name="tb", tag="tb")
                nc.scalar.activation(
                    out=ta[:].rearrange("r p e -> r (p e)").rearrange(
                        "r (j x) -> r j x", j=2),
                    in_=ps_a[:, :, 0:384], func=Relu,
                    bias=bt[:, 2 * q:2 * q + 1], scale=1.0)
                nc.scalar.activation(
                    out=tb[:].rearrange("r p e -> r (p e)").rearrange(
                        "r (j x) -> r j x", j=2),
                    in_=ps_b[:, :, 0:384], func=Relu,
                    bias=bt[:, 2 * q + 1:2 * q + 2], scale=1.0)

                # products: p_0 = pos0*pos2, p_1 = pos1*pos2 on DVE,
                # p_2 = pos1*pos0 on GPSIMD (latency hidden by the pipeline)
                pa = work.tile([128, 3, 256], F16, name="pa", tag="pa")
                pb = work.tile([128, 3, 256], F16, name="pb", tag="pb")
                for t_, p_ in ((ta, pa), (tb, pb)):
                    nc.vector.tensor_mul(out=p_[:, 0, :], in0=t_[:, 0, :],
                                         in1=t_[:, 2, :])
                    nc.vector.tensor_mul(out=p_[:, 1, :], in0=t_[:, 1, :],
                                         in1=t_[:, 2, :])
                    nc.gpsimd.tensor_mul(out=p_[:, 2, :], in0=t_[:, 1, :],
                                         in1=t_[:, 0, :])

                if prev is not None:
                    emit_back(prev)
                prev = (pkt, pa, pb, q)
            emit_back(prev)
    nc.finalize()
    return nc


def _get_program(P):
    if P not in _prog_cache:
        _prog_cache[P] = _build_program(P)
    return _prog_cache[P]


def _prepare(x, nodes, fact, params, bias_p, ho_params, ho_bias):
    """Host-side: sort by id, build per-pair packed fp16 arrays."""
    N, L = nodes.shape
    E = fact.shape[0]
    R = params.shape[2]
    NP = params.shape[0]           # 169
    MA = int(round(NP ** 0.5))     # 13

    ids = (x[fact[:, 0], 1] * MA + x[fact[:, 0], 2]).astype(np.int64)   # [E]
    perm = np.argsort(ids, kind="stable")
    ids_s = ids[perm]
    fact_s = fact[perm].astype(np.int64)                                 # [E,3]

    counts = np.bincount(ids_s, minlength=NP)                            # [NP]
    nblk = (counts + _BLK - 1) // _BLK                                   # [NP]
    blk_ids = np.repeat(np.arange(NP), nblk)                             # [NB]
    NB = int(blk_ids.shape[0])
    B = (NB + _NCORES - 1) // _NCORES
    if B % 2:
        B += 1
    NB8 = B * _NCORES
    blk_ids = np.concatenate([blk_ids, np.zeros(NB8 - NB, np.int64)])

    # slot -> sorted-edge-position map (-1 = padding)
    padded = nblk * _BLK
    pad_off = np.concatenate([[0], np.cumsum(padded)])
    off = np.concatenate([[0], np.cumsum(counts)])
    total = int(pad_off[-1])
    t_of = np.repeat(np.arange(NP), padded)
    jloc = np.arange(total) - pad_off[t_of]
    src = np.where(jloc < counts[t_of], off[t_of] + jloc, -1)
    src = np.concatenate([src, np.full(NB8 * _BLK - total, -1, np.int64)])
    valid = src >= 0

    # gather features per slot
    nf = nodes[fact_s].astype(np.float16)                                # [E,3,L]
    featp = np.zeros((NB8 * _BLK, 3, L), np.float16)
    featp[valid] = nf[src[valid]]

    NPAIR = NB8 // 2
    # pkin: cols 0:768 feats, 768:896 W, 896:1280 ho
    pkin = np.zeros((NPAIR, 128, 1280), np.float16)
    # feats: [q, 64c+l, pos*256 + seg*128 + e], storage order pos = (1, 0, 2)
    pkin[:, :, 0:768] = (
        featp[:, [1, 0, 2], :]                      # slot pos l
        .reshape(NPAIR, 2, 2, _SEG, 3, L)           # q c seg e pos l
        .transpose(0, 1, 5, 4, 2, 3)                # q c l pos seg e
        .reshape(NPAIR, 128, 768))
    # W: rows 0:64 = W_a, 64:128 = W_b
    pkin[:, :, 768:896] = (
        params[blk_ids].astype(np.float16)          # [NB8, L, R]
        .reshape(NPAIR, 2 * L, R))
    # ho: [q, r, 192c + 64i + l]
    pkin[:, :, 896:1280] = (
        ho_params[:, blk_ids].astype(np.float16)    # [3, NB8, R, L]
        .transpose(1, 2, 0, 3)                      # NB8 r i l
        .reshape(NPAIR, 2, R, 3 * L)                # q c r (i l)
        .transpose(0, 2, 1, 3)                      # q r c (i l)
        .reshape(NPAIR, R, 384))

    biasT = bias_p[blk_ids, 0].astype(np.float32)                        # [NB8,R]
    biasT = biasT.reshape(_NCORES, B, R).transpose(0, 2, 1)              # [8,R,B]

    return dict(pkin=pkin, biasT=np.ascontiguousarray(biasT),
                B=B, NB8=NB8, P=B // 2,
                src=src, valid=valid, fact_s=fact_s, ids_s=ids_s,
                N=N, E=E, L=L)


def _postprocess(msgs_all, prep, ho_bias):
    """Decode per-slot messages, add host-side b2, segment-sum into node_msg."""
    NB8, N, E, L = prep["NB8"], prep["N"], prep["E"], prep["L"]
    src, valid, fact_s, ids_s = prep["src"], prep["valid"], prep["fact_s"], prep["ids_s"]
    NPAIR = NB8 // 2
    # msgs_all [NPAIR, 128, 768]: partition 64c+l, col = i*256 + seg*128 + e
    slots = (
        msgs_all.astype(np.float32)
        .reshape(NPAIR, 2, 64, 3, 2, _SEG)          # q c l i seg e
        .transpose(0, 1, 4, 5, 3, 2)                # q c seg e i l
        .reshape(NB8 * _BLK, 3, 64)
    )
    msg_e = np.empty((E, 3, L), np.float32)
    msg_e[src[valid]] = slots[valid]

    # fold in the second bias (linear in the segment-sum)
    msg_e += ho_bias[:, ids_s, 0].astype(np.float32).transpose(1, 0, 2)  # [E,3,L]

    idx_all = fact_s.T.reshape(-1)                                       # [3E]
    val_all = msg_e.transpose(1, 0, 2).reshape(-1, L)                    # [3E,L]
    order = np.argsort(idx_all, kind="stable")
    idx_sorted = idx_all[order]
    val_sorted = val_all[order]
    uniq, starts = np.unique(idx_sorted, return_index=True)
    sums = np.add.reduceat(val_sorted, starts, axis=0)
    out = np.zeros((N, L), np.float32)
    out[uniq] = sums
    return out


def _run_device(prep, trace=False, trace_kwargs=None):
    from concourse.bass_utils import run_bass_kernel_spmd

    P = prep["P"]
    nc = _get_program(P)
    in_maps = []
    for c in range(_NCORES):
        in_maps.append({
            "pkin": prep["pkin"][c * P:(c + 1) * P],
            "bia": prep["biasT"][c],
        })
    kwargs = {}
    if trace:
        kwargs["trace"] = True
        if trace_kwargs:
            kwargs.update(trace_kwargs)
    res = run_bass_kernel_spmd(nc, in_maps, list(range(_NCORES)), **kwargs)
    msgs_all = np.concatenate([res.results[c]["msgs"] for c in range(_NCORES)],
                              axis=0)
    return msgs_all, res


def kernel(x, nodes, fact, fact_dim, params, bias_p, ho_params, ho_bias,
           _trace=False, _trace_kwargs=None):
    x = np.asarray(x)
    nodes = np.asarray(nodes, dtype=np.float32)
    fact = np.asarray(fact)
    params = np.asarray(params)
    bias_p = np.asarray(bias_p)
    ho_params = np.asarray(ho_params)
    ho_bias = np.asarray(ho_bias)

    prep = _prepare(x, nodes, fact, params, bias_p, ho_params, ho_bias)
    msgs_all, res = _run_device(prep, trace=_trace, trace_kwargs=_trace_kwargs)
    out = _postprocess(msgs_all, prep, ho_bias)
    kernel.last_results = res
    return out


# revision 21
# speedup vs baseline: 1.1765x; 1.0377x over previous
"""Trainium2 Bass kernel for nn_FGNet (gnn_message_passing) — v3.6.

Strategy
--------
Edges sorted by type id, packed into per-type blocks of 256 edges (2 segments
x 128) or 128 edges (1 segment, for small remainders), processed in same-size
PAIRS (block a on SBUF partitions 0:64, block b on 64:128).  All device
tensors fp16 (PSUM stays f32); rel err ~3e-4 vs the 2e-2 gate.

Per 256-pair q:
    mm1 (K=64, row-tiled): T(0,0) computes W_a.T @ feats_a, T(64,0) computes
        W_b.T @ feats_b; N=384 halves, each inside one 2KB PSUM bank
    relu+bias (ACT): psum f32 -> t fp16, one 3D-AP instruction per block
    products (DVE x2 + GPSIMD x1 per block): p_i = t_j * t_k, fp16
    mm2 (M=64, col-tiled): T(0,0) -> psum partitions 0:64 (block a),
        T(0,64) -> 64:128 (block b); natural [128,768] pair layout
    evac: ACT copy 1/3 + DVE casts 2/3 -> m fp16; DMA out
    (second bias b2 is linear in the segment-sum -> folded to the host)
128-pairs are the same with half the edge columns (one segment).

Software pipelining: mm2/evac/store of pair q-1 are emitted after products(q)
so the in-order PE queue never waits on same-pair products.

HW constraints baked in (validated on this stack, see mb*.py probes):
  - matmul out must lie inside one 2KB PSUM bank; N=512 f32 output is
    silently WRONG on HW -> N<=448; we use 384
  - two row-tiles (T0/T8) must never write the same PSUM bank
  - DMA and GPSIMD cannot touch PSUM; hwdge queues = {sync, scalar} only
  - big DVE ops pay a pipe-drain tax -> prefer several small ops
  - fp16 K=64 matmuls + tile_position row/col tiling work (f32r K=64 broken)

Packed input per pair (host-side numpy):
  256-pair pkin[q] [128, 1280] fp16:
    cols 0:768  feats   partition 64c+l, col = pos*256 + seg*128 + e,
                        t storage order pos = (i1, i0, i2)
    cols 768:896  W     rows 0:64 = W_a, 64:128 = W_b
    cols 896:1280 ho    col 896 + 192c + 64i + l = ho_params[i, id_c, r, l]
  128-pair pkin128[q] [128, 896]: feats 0:384 (col = pos*128 + e),
    W 384:512, ho 512:896 (col 512 + 192c + 64i + l)
Output msgs[q] [128, 768]: partition 64c+l, col = i*256 + seg*128 + e
       msgs128[q] [128, 384]: col = i*128 + e.

Host side: id computation, sort, feature gather, packing, unpermute, b2 bias
add and the final segment-sum into node_msg.
"""

import numpy as np

_SEG = 128
_NCORES = 8

_prog_cache = {}


def _build_program(P256, P128):
    """SPMD device program: P256 256-pairs then P128 128-pairs per core."""
    import concourse.mybir as mybir
    import concourse.tile as tile
    from concourse import bacc

    F32 = mybir.dt.float32
    F16 = mybir.dt.float16
    Relu = mybir.ActivationFunctionType.Relu
    Copy = mybir.ActivationFunctionType.Copy

    B = 2 * (P256 + P128)
    nc = bacc.Bacc()
    pkin = nc.declare_dram_parameter("pkin", [max(P256, 1), 128, 1280], F16,
                                     isOutput=False)
    pkin128 = nc.declare_dram_parameter("pkin128", [max(P128, 1), 128, 896],
                                        F16, isOutput=False)
    bia = nc.declare_dram_parameter("bia", [128, B], F32, isOutput=False)
    msgs = nc.declare_dram_parameter("msgs", [max(P256, 1), 128, 768], F16,
                                     isOutput=True)
    msgs128 = nc.declare_dram_parameter("msgs128", [max(P128, 1), 128, 384],
                                        F16, isOutput=True)

    with tile.TileContext(nc) as tc:
        with (
            tc.tile_pool(name="const", bufs=1) as const,
            tc.tile_pool(name="work", bufs=6) as work,
            tc.tile_pool(name="psA", bufs=1, space="PSUM") as psA,
            tc.tile_pool(name="psB", bufs=2, space="PSUM") as psB,
        ):
            bt = const.tile([128, B], F32, name="bt")
            nc.sync.dma_start(out=bt[:], in_=bia[:])

            def emit_back(prev):
                # software-pipelined tail of the previous pair
                pkt, pa, pb, q, big = prev
                S = 256 if big else 128          # edge cols per (block, i)
                ho0 = 896 if big else 512
                paf = pa[:].rearrange("r p e -> r (p e)")
                pbf = pb[:].rearrange("r p e -> r (p e)")
                ps2 = psB.tile([128, 3, 256], F32, name="ps2", tag="ps2")
                for i in range(3):
                    nc.tensor.matmul(
                        out=ps2[0:64, i, 0:S],
                        lhsT=pkt[:, ho0 + 64 * i:ho0 + 64 * (i + 1)],
                        rhs=paf[:, i * S:(i + 1) * S],
                        start=True, stop=True, tile_position=(0, 0))
                    nc.tensor.matmul(
                        out=ps2[64:128, i, 0:S],
                        lhsT=pkt[:, ho0 + 192 + 64 * i:ho0 + 192 + 64 * (i + 1)],
                        rhs=pbf[:, i * S:(i + 1) * S],
                        start=True, stop=True, tile_position=(0, 64))
                m = work.tile([128, 768], F16, name="m", tag="m")
                if big:
                    ps2f = ps2[:].rearrange("l i c -> l (i c)")
                    nc.scalar.activation(out=m[:, 0:256], in_=ps2f[:, 0:256],
                                         func=Copy, bias=0.0, scale=1.0)
                    nc.vector.tensor_copy(out=m[:, 256:512],
                                          in_=ps2f[:, 256:512])
                    nc.vector.tensor_copy(out=m[:, 512:768],
                                          in_=ps2f[:, 512:768])
                    nc.sync.dma_start(out=msgs[q], in_=m[:])
                else:
                    nc.scalar.activation(out=m[:, 0:128],
                                         in_=ps2[:, 0, 0:128],
                                         func=Copy, bias=0.0, scale=1.0)
                    nc.vector.tensor_copy(
                        out=m[:, 128:384].rearrange("l (i c) -> l i c", i=2),
                        in_=ps2[:, 1:3, 0:128])
                    nc.sync.dma_start(out=msgs128[q], in_=m[:, 0:384])

            def emit_pair(q, big, bcol):
                # front half of pair q: load, mm1, relu, products
                S = 256 if big else 128
                NC = 1280 if big else 896
                w0 = 768 if big else 384
                src = pkin[q] if big else pkin128[q]
                pkt = work.tile([128, 1280], F16, name="pkt", tag="pkt")
                nc.sync.dma_start(out=pkt[:, 0:NC], in_=src)
                fkt = pkt[:, 0:3 * S]
                wkt = pkt[:, w0:w0 + 128]

                # mm1: row-tiled K=64, halves of 3*S/2 cols per bank
                H = 3 * S // 2                  # 384 or 192
                ps_a = psA.tile([128, 2, 512], F32, name="ps_a", tag="ps_a")
                ps_b = psA.tile([128, 2, 512], F32, name="ps_b", tag="ps_b")
                for j in range(2):
                    nc.tensor.matmul(
                        out=ps_a[:, j, 0:H], lhsT=wkt[0:64, :],
                        rhs=fkt[0:64, H * j:H * (j + 1)],
                        start=True, stop=True, tile_position=(0, 0))
                    nc.tensor.matmul(
                        out=ps_b[:, j, 0:H], lhsT=wkt[64:128, :],
                        rhs=fkt[64:128, H * j:H * (j + 1)],
                        start=True, stop=True, tile_position=(64, 0))

                # relu+bias on ACT; t cols pos-major (pos = t_1, t_0, t_2),
                # packed contiguously: pos p at flat cols [p*S, (p+1)*S)
                ta = work.tile([128, 3, 256], F16, name="ta", tag="ta")
                tb = work.tile([128, 3, 256], F16, name="tb", tag="tb")
                for t_, ps_, col in ((ta, ps_a, bcol), (tb, ps_b, bcol + 1)):
                    tf = t_[:].rearrange("r p e -> r (p e)")
                    nc.scalar.activation(
                        out=tf[:, 0:3 * S].rearrange("r (j x) -> r j x", j=2),
                        in_=ps_[:, :, 0:H], func=Relu,
                        bias=bt[:, col:col + 1], scale=1.0)

                # products: p_0 = pos0*pos2, p_1 = pos1*pos2 (DVE),
                # p_2 = pos1*pos0 (GPSIMD, latency hidden by the pipeline);
                # p_i packed contiguously at flat cols [i*S, (i+1)*S)
                pa = work.tile([128, 3, 256], F16, name="pa", tag="pa")
                pb = work.tile([128, 3, 256], F16, name="pb", tag="pb")
                for t_, p_ in ((ta, pa), (tb, pb)):
                    tf = t_[:].rearrange("r p e -> r (p e)")
                    pf = p_[:].rearrange("r p e -> r (p e)")
                    nc.vector.tensor_mul(out=pf[:, 0:S], in0=tf[:, 0:S],
                                         in1=tf[:, 2 * S:3 * S])
                    nc.vector.tensor_mul(out=pf[:, S:2 * S], in0=tf[:, S:2 * S],
                                         in1=tf[:, 2 * S:3 * S])
                    nc.gpsimd.tensor_mul(out=pf[:, 2 * S:3 * S],
                                         in0=tf[:, S:2 * S], in1=tf[:, 0:S])
                return (pkt, pa, pb, q, big)

            prev = None
            for q in range(P256):
                cur = emit_pair(q, True, 2 * q)
                if prev is not None:
                    emit_back(prev)
                prev = cur
            for q in range(P128):
                cur = emit_pair(q, False, 2 * P256 + 2 * q)
                if prev is not None:
                    emit_back(prev)
                prev = cur
            if prev is not None:
                emit_back(prev)
    nc.finalize()
    return nc


def _get_program(P256, P128):
    key = (P256, P128)
    if key not in _prog_cache:
        _prog_cache[key] = _build_program(P256, P128)
    return _prog_cache[key]


def _plan_blocks(counts):
    """Per type: n256 256-blocks and optionally one 128-block."""
    NP = counts.shape[0]
    blocks = []                     # (type, size)
    for t in range(NP):
        c = int(counts[t])
        n256, r = divmod(c, 256)
        if r > 128 or (r == 0 and c > 0 and n256 == 0):
            n256 += 1
            r = 0
        for _ in range(n256):
            blocks.append((t, 256))
        if r > 0:
            blocks.append((t, 128))
    b256 = [t for t, s in blocks if s == 256]
    b128 = [t for t, s in blocks if s == 128]
    # SPMD uniformity: equal even per-core counts; upgrade overflow 128s
    B128 = (len(b128) // (2 * _NCORES)) * 2     # round DOWN to even per core
    while len(b128) > B128 * _NCORES:
        b256.append(b128.pop())                 # upgrade to a half-empty 256
    B256 = -(-len(b256) // (2 * _NCORES)) * 2   # round UP to even per core
    b256 += [0] * (B256 * _NCORES - len(b256))  # pad blocks (type 0, empty)
    return b256, b128, B256, B128


def _prepare(x, nodes, fact, params, bias_p, ho_params, ho_bias):
    """Host-side: sort by id, plan blocks, build packed fp16 arrays."""
    N, L = nodes.shape
    E = fact.shape[0]
    R = params.shape[2]
    NP = params.shape[0]           # 169
    MA = int(round(NP ** 0.5))     # 13

    ids = (x[fact[:, 0], 1] * MA + x[fact[:, 0], 2]).astype(np.int64)   # [E]
    perm = np.argsort(ids, kind="stable")
    ids_s = ids[perm]
    fact_s = fact[perm].astype(np.int64)                                 # [E,3]
    counts = np.bincount(ids_s, minlength=NP)                            # [NP]
    off = np.concatenate([[0], np.cumsum(counts)])

    b256, b128, B256, B128 = _plan_blocks(counts)
    NB256, NB128 = len(b256), len(b128)
    P256, P128 = B256 // 2, B128 // 2

    # slot -> sorted-edge-position map (-1 = padding).  256-blocks occupy
    # slots [0, NB256*256); 128-blocks follow.  Per type, its blocks appear
    # in list order (all 256s then its 128), matching edge order.
    tslots = NB256 * 256 + NB128 * 128
    src = np.full(tslots, -1, np.int64)
    used = np.zeros(NP, np.int64)
    cur = {}
    for b, t in enumerate(b256):
        cur.setdefault(t, []).append((b * 256, 256))
    for b, t in enumerate(b128):
        cur.setdefault(t, []).append((NB256 * 256 + b * 128, 128))
    for t, regions in cur.items():
        have = int(counts[t])
        taken = 0
        for start, size in regions:
            n = min(size, have - taken)
            if n <= 0:
                break
            src[start:start + n] = np.arange(off[t] + taken,
                                             off[t] + taken + n)
            taken += n
    valid = src >= 0

    nf = nodes[fact_s].astype(np.float16)                                # [E,3,L]
    featp = np.zeros((tslots, 3, L), np.float16)
    featp[valid] = nf[src[valid]]
    featp = featp[:, [1, 0, 2], :]          # storage order pos = (i1, i0, i2)

    blk_ids = np.array(b256 + b128, np.int64)                # [NB256+NB128]
    Wb = params[blk_ids].astype(np.float16)                  # [NB, L, R]
    HOb = (ho_params[:, blk_ids].astype(np.float16)          # [3, NB, R, L]
           .transpose(1, 2, 0, 3).reshape(-1, R, 3 * L))     # [NB, R, 192]

    NPAIR256, NPAIR128 = NB256 // 2, NB128 // 2
    pkin = np.zeros((max(NPAIR256, 1), 128, 1280), np.float16)
    if NPAIR256:
        pkin[:, :, 0:768] = (
            featp[0:NB256 * 256]
            .reshape(NPAIR256, 2, 2, _SEG, 3, L)   # q c seg e pos l
            .transpose(0, 1, 5, 4, 2, 3)           # q c l pos seg e
            .reshape(NPAIR256, 128, 768))
        pkin[:, :, 768:896] = Wb[0:NB256].reshape(NPAIR256, 2 * L, R)
        pkin[:, :, 896:1280] = (
            HOb[0:NB256].reshape(NPAIR256, 2, R, 192)
            .transpose(0, 2, 1, 3).reshape(NPAIR256, R, 384))
    pkin128 = np.zeros((max(NPAIR128, 1), 128, 896), np.float16)
    if NPAIR128:
        pkin128[:, :, 0:384] = (
            featp[NB256 * 256:]
            .reshape(NPAIR128, 2, _SEG, 3, L)      # q c e pos l
            .transpose(0, 1, 4, 3, 2)              # q c l pos e
            .reshape(NPAIR128, 128, 384))
        pkin128[:, :, 384:512] = Wb[NB256:].reshape(NPAIR128, 2 * L, R)
        pkin128[:, :, 512:896] = (
            HOb[NB256:].reshape(NPAIR128, 2, R, 192)
            .transpose(0, 2, 1, 3).reshape(NPAIR128, R, 384))

    # bias per block, core-major column order: 256-blocks then 128-blocks
    bias_blk = bias_p[blk_ids, 0].astype(np.float32)          # [NB, R]
    biasT = np.zeros((_NCORES, R, B256 + B128), np.float32)
    if NB256:
        biasT[:, :, 0:B256] = (bias_blk[0:NB256]
                               .reshape(_NCORES, B256, R).transpose(0, 2, 1))
    if NB128:
        biasT[:, :, B256:] = (bias_blk[NB256:]
                              .reshape(_NCORES, B128, R).transpose(0, 2, 1))

    return dict(pkin=pkin, pkin128=pkin128, biasT=np.ascontiguousarray(biasT),
                P256=P256, P128=P128, NB256=NB256, NB128=NB128,
                src=src, valid=valid, fact_s=fact_s, ids_s=ids_s,
                N=N, E=E, L=L, tslots=tslots)


def _postprocess(msgs_all, msgs128_all, prep, ho_bias):
    """Decode per-slot messages, add host-side b2, segment-sum into node_msg."""
    N, E, L = prep["N"], prep["E"], prep["L"]
    NB256, NB128 = prep["NB256"], prep["NB128"]
    src, valid, fact_s, ids_s = (prep["src"], prep["valid"],
                                 prep["fact_s"], prep["ids_s"])
    parts = []
    if NB256:
        parts.append(
            msgs_all.astype(np.float32)
            .reshape(NB256 // 2, 2, 64, 3, 2, _SEG)   # q c l i seg e
            .transpose(0, 1, 4, 5, 3, 2)              # q c seg e i l
            .reshape(NB256 * 256, 3, 64))
    if NB128:
        parts.append(
            msgs128_all.astype(np.float32)
            .reshape(NB128 // 2, 2, 64, 3, _SEG)      # q c l i e
            .transpose(0, 1, 4, 3, 2)                 # q c e i l
            .reshape(NB128 * 128, 3, 64))
    slots = np.concatenate(parts, axis=0)

    msg_e = np.empty((E, 3, L), np.float32)
    msg_e[src[valid]] = slots[valid]
    msg_e += ho_bias[:, ids_s, 0].astype(np.float32).transpose(1, 0, 2)  # [E,3,L]

    idx_all = fact_s.T.reshape(-1)                                       # [3E]
    val_all = msg_e.transpose(1, 0, 2).reshape(-1, L)                    # [3E,L]
    order = np.argsort(idx_all, kind="stable")
    idx_sorted = idx_all[order]
    val_sorted = val_all[order]
    uniq, starts = np.unique(idx_sorted, return_index=True)
    sums = np.add.reduceat(val_sorted, starts, axis=0)
    out = np.zeros((N, L), np.float32)
    out[uniq] = sums
    return out


def _run_device(prep, trace=False, trace_kwargs=None):
    from concourse.bass_utils import run_bass_kernel_spmd

    P256, P128 = prep["P256"], prep["P128"]
    nc = _get_program(P256, P128)
    in_maps = []
    for c in range(_NCORES):
        in_maps.append({
            "pkin": prep["pkin"][c * P256:(c + 1) * P256] if P256
            else prep["pkin"],
            "pkin128": prep["pkin128"][c * P128:(c + 1) * P128] if P128
            else prep["pkin128"],
            "bia": prep["biasT"][c],
        })
    kwargs = {}
    if trace:
        kwargs["trace"] = True
        if trace_kwargs:
            kwargs.update(trace_kwargs)
    res = run_bass_kernel_spmd(nc, in_maps, list(range(_NCORES)), **kwargs)
    msgs_all = np.concatenate([res.results[c]["msgs"] for c in range(_NCORES)],
                              axis=0) if P256 else None
    msgs128_all = np.concatenate([res.results[c]["msgs128"]
                                  for c in range(_NCORES)], axis=0) if P128 \
        else None
    return msgs_all, msgs128_all, res


def kernel(x, nodes, fact, fact_dim, params, bias_p, ho_params, ho_bias,
           _trace=False, _trace_kwargs=None):
    x = np.asarray(x)
    nodes = np.asarray(nodes, dtype=np.float32)
    fact = np.asarray(fact)
    params = np.asarray(params)
    bias_p = np.asarray(bias_p)
    ho_params = np.asarray(ho_params)
    ho_bias = np.asarray(ho_bias)

    prep = _prepare(x, nodes, fact, params, bias_p, ho_params, ho_bias)
    msgs_all, msgs128_all, res = _run_device(prep, trace=_trace,
                                             trace_kwargs=_trace_kwargs)
    out = _postprocess(msgs_all, msgs128_all, prep, ho_bias)
    kernel.last_results = res
    return out


# revision 24
# speedup vs baseline: 1.2117x; 1.0299x over previous
"""Trainium2 Bass kernel for nn_FGNet (gnn_message_passing) — v3.6.

Strategy
--------
Edges sorted by type id, packed into per-type blocks of 256 edges (2 segments
x 128) or 128 edges (1 segment, for small remainders), processed in same-size
PAIRS (block a on SBUF partitions 0:64, block b on 64:128).  All device
tensors fp16 (PSUM stays f32); rel err ~3e-4 vs the 2e-2 gate.

Per 256-pair q:
    mm1 (K=64, row-tiled): T(0,0) computes W_a.T @ feats_a, T(64,0) computes
        W_b.T @ feats_b; N=384 halves, each inside one 2KB PSUM bank
    relu+bias (ACT): psum f32 -> t fp16, one 3D-AP instruction per block
    products (DVE x2 + GPSIMD x1 per block): p_i = t_j * t_k, fp16
    mm2 (M=64, col-tiled): T(0,0) -> psum partitions 0:64 (block a),
        T(0,64) -> 64:128 (block b); natural [128,768] pair layout
    evac: ACT copy 1/3 + DVE casts 2/3 -> m fp16; DMA out
    (second bias b2 is linear in the segment-sum -> folded to the host)
128-pairs are the same with half the edge columns (one segment).

Software pipelining: mm2/evac/store of pair q-1 are emitted after products(q)
so the in-order PE queue never waits on same-pair products.

HW constraints baked in (validated on this stack, see mb*.py probes):
  - matmul out must lie inside one 2KB PSUM bank; N=512 f32 output is
    silently WRONG on HW -> N<=448; we use 384
  - two row-tiles (T0/T8) must never write the same PSUM bank
  - DMA and GPSIMD cannot touch PSUM; hwdge queues = {sync, scalar} only
  - big DVE ops pay a pipe-drain tax -> prefer several small ops
  - fp16 K=64 matmuls + tile_position row/col tiling work (f32r K=64 broken)

Packed input per pair (host-side numpy):
  256-pair pkin[q] [128, 1280] fp16:
    cols 0:768  feats   partition 64c+l, col = pos*256 + seg*128 + e,
                        t storage order pos = (i1, i0, i2)
    cols 768:896  W     rows 0:64 = W_a, 64:128 = W_b
    cols 896:1280 ho    col 896 + 192c + 64i + l = ho_params[i, id_c, r, l]
  128-pair pkin128[q] [128, 896]: feats 0:384 (col = pos*128 + e),
    W 384:512, ho 512:896 (col 512 + 192c + 64i + l)
Output msgs[q] [128, 768]: partition 64c+l, col = i*256 + seg*128 + e
       msgs128[q] [128, 384]: col = i*128 + e.

Host side: id computation, sort, feature gather, packing, unpermute, b2 bias
add and the final segment-sum into node_msg.
"""

import numpy as np

_SEG = 128
_NCORES = 8

_prog_cache = {}


def _build_program(P256, P128):
    """SPMD device program: P256 256-pairs then P128 128-pairs per core."""
    import concourse.mybir as mybir
    import concourse.tile as tile
    from concourse import bacc

    F32 = mybir.dt.float32
    F16 = mybir.dt.float16
    Relu = mybir.ActivationFunctionType.Relu
    Copy = mybir.ActivationFunctionType.Copy

    B = 2 * (P256 + P128)
    nc = bacc.Bacc()
    pkin = nc.declare_dram_parameter("pkin", [max(P256, 1), 128, 1280], F16,
                                     isOutput=False)
    pkin128 = nc.declare_dram_parameter("pkin128", [max(P128, 1), 128, 896],
                                        F16, isOutput=False)
    bia = nc.declare_dram_parameter("bia", [128, B], F32, isOutput=False)
    msgs = nc.declare_dram_parameter("msgs", [max(P256, 1), 128, 768], F16,
                                     isOutput=True)
    msgs128 = nc.declare_dram_parameter("msgs128", [max(P128, 1), 128, 384],
                                        F16, isOutput=True)

    with tile.TileContext(nc) as tc:
        with (
            tc.tile_pool(name="const", bufs=1) as const,
            tc.tile_pool(name="work", bufs=6) as work,
            tc.tile_pool(name="psA", bufs=1, space="PSUM") as psA,
            tc.tile_pool(name="psB", bufs=2, space="PSUM") as psB,
        ):
            bt = const.tile([128, B], F32, name="bt")
            nc.scalar.dma_start(out=bt[:], in_=bia[:])

            def emit_back(prev):
                # software-pipelined tail of the previous pair
                pkt, pa, pb, q, big = prev
                S = 256 if big else 128          # edge cols per (block, i)
                ho0 = 896 if big else 512
                paf = pa[:].rearrange("r p e -> r (p e)")
                pbf = pb[:].rearrange("r p e -> r (p e)")
                ps2 = psB.tile([128, 3, 256], F32, name="ps2", tag="ps2")
                for i in range(3):
                    nc.tensor.matmul(
                        out=ps2[0:64, i, 0:S],
                        lhsT=pkt[:, ho0 + 64 * i:ho0 + 64 * (i + 1)],
                        rhs=paf[:, i * S:(i + 1) * S],
                        start=True, stop=True, tile_position=(0, 0))
                    nc.tensor.matmul(
                        out=ps2[64:128, i, 0:S],
                        lhsT=pkt[:, ho0 + 192 + 64 * i:ho0 + 192 + 64 * (i + 1)],
                        rhs=pbf[:, i * S:(i + 1) * S],
                        start=True, stop=True, tile_position=(0, 64))
                m = work.tile([128, 768], F16, name="m", tag="m")
                if big:
                    ps2f = ps2[:].rearrange("l i c -> l (i c)")
                    nc.vector.tensor_copy(out=m[:, 0:384], in_=ps2f[:, 0:384])
                    nc.vector.tensor_copy(out=m[:, 384:768],
                                          in_=ps2f[:, 384:768])
                    if q == P256 - 1 and P128 == 0:
                        nc.sync.dma_start(out=msgs[q][:, 0:384],
                                          in_=m[:, 0:384])
                        nc.scalar.dma_start(out=msgs[q][:, 384:768],
                                            in_=m[:, 384:768])
                    else:
                        nc.sync.dma_start(out=msgs[q], in_=m[:])
                else:
                    nc.vector.tensor_copy(
                        out=m[:, 0:384].rearrange("l (i c) -> l i c", i=3),
                        in_=ps2[:, :, 0:128])
                    if q == P128 - 1:
                        nc.sync.dma_start(out=msgs128[q][:, 0:192],
                                          in_=m[:, 0:192])
                        nc.scalar.dma_start(out=msgs128[q][:, 192:384],
                                            in_=m[:, 192:384])
                    else:
                        nc.sync.dma_start(out=msgs128[q], in_=m[:, 0:384])

            def emit_pair(q, big, bcol):
                # front half of pair q: load, mm1, relu, products
                S = 256 if big else 128
                NC = 1280 if big else 896
                w0 = 768 if big else 384
                src = pkin[q] if big else pkin128[q]
                pkt = work.tile([128, 1280], F16, name="pkt", tag="pkt")
                if big and q == 0:
                    # split the first load across both hwdge queues: the
                    # ramp is latency-bound on this transfer
                    nc.sync.dma_start(out=pkt[:, 0:640], in_=src[:, 0:640])
                    nc.scalar.dma_start(out=pkt[:, 640:NC], in_=src[:, 640:NC])
                else:
                    nc.sync.dma_start(out=pkt[:, 0:NC], in_=src)
                fkt = pkt[:, 0:3 * S]
                wkt = pkt[:, w0:w0 + 128]

                # mm1: row-tiled K=64, halves of 3*S/2 cols per bank
                H = 3 * S // 2                  # 384 or 192
                ps_a = psA.tile([128, 2, 512], F32, name="ps_a", tag="ps_a")
                ps_b = psA.tile([128, 2, 512], F32, name="ps_b", tag="ps_b")
                for j in range(2):
                    nc.tensor.matmul(
                        out=ps_a[:, j, 0:H], lhsT=wkt[0:64, :],
                        rhs=fkt[0:64, H * j:H * (j + 1)],
                        start=True, stop=True, tile_position=(0, 0))
                    nc.tensor.matmul(
                        out=ps_b[:, j, 0:H], lhsT=wkt[64:128, :],
                        rhs=fkt[64:128, H * j:H * (j + 1)],
                        start=True, stop=True, tile_position=(64, 0))

                # relu+bias on ACT; t cols pos-major (pos = t_1, t_0, t_2),
                # packed contiguously: pos p at flat cols [p*S, (p+1)*S)
                ta = work.tile([128, 3, 256], F16, name="ta", tag="ta")
                tb = work.tile([128, 3, 256], F16, name="tb", tag="tb")
                for t_, ps_, col in ((ta, ps_a, bcol), (tb, ps_b, bcol + 1)):
                    tf = t_[:].rearrange("r p e -> r (p e)")
                    nc.scalar.activation(
                        out=tf[:, 0:3 * S].rearrange("r (j x) -> r j x", j=2),
                        in_=ps_[:, :, 0:H], func=Relu,
                        bias=bt[:, col:col + 1], scale=1.0)

                # products: p_0 = pos0*pos2, p_1 = pos1*pos2 (DVE),
                # p_2 = pos1*pos0 (GPSIMD, latency hidden by the pipeline);
                # p_i packed contiguously at flat cols [i*S, (i+1)*S)
                pa = work.tile([128, 3, 256], F16, name="pa", tag="pa")
                pb = work.tile([128, 3, 256], F16, name="pb", tag="pb")
                for t_, p_ in ((ta, pa), (tb, pb)):
                    tf = t_[:].rearrange("r p e -> r (p e)")
                    pf = p_[:].rearrange("r p e -> r (p e)")
                    nc.vector.tensor_mul(out=pf[:, 0:S], in0=tf[:, 0:S],
                                         in1=tf[:, 2 * S:3 * S])
                    nc.vector.tensor_mul(out=pf[:, S:2 * S], in0=tf[:, S:2 * S],
                                         in1=tf[:, 2 * S:3 * S])
                    nc.gpsimd.tensor_mul(out=pf[:, 2 * S:3 * S],
                                         in0=tf[:, S:2 * S], in1=tf[:, 0:S])
                return (pkt, pa, pb, q, big)

            prev = None
            for q in range(P256):
                cur = emit_pair(q, True, 2 * q)
                if prev is not None:
                    emit_back(prev)
                prev = cur
            for q in range(P128):
                cur = emit_pair(q, False, 2 * P256 + 2 * q)
                if prev is not None:
                    emit_back(prev)
                prev = cur
            if prev is not None:
                emit_back(prev)
    nc.finalize()
    return nc


def _get_program(P256, P128):
    key = (P256, P128)
    if key not in _prog_cache:
        _prog_cache[key] = _build_program(P256, P128)
    return _prog_cache[key]


def _plan_blocks(counts):
    """Per type: n256 256-blocks and optionally one 128-block."""
    NP = counts.shape[0]
    blocks = []                     # (type, size)
    for t in range(NP):
        c = int(counts[t])
        n256, r = divmod(c, 256)
        if r > 128 or (r == 0 and c > 0 and n256 == 0):
            n256 += 1
            r = 0
        for _ in range(n256):
            blocks.append((t, 256))
        if r > 0:
            blocks.append((t, 128))
    b256 = [t for t, s in blocks if s == 256]
    b128 = [t for t, s in blocks if s == 128]
    # SPMD uniformity: equal even per-core counts; upgrade overflow 128s
    B128 = (len(b128) // (2 * _NCORES)) * 2     # round DOWN to even per core
    while len(b128) > B128 * _NCORES:
        b256.append(b128.pop())                 # upgrade to a half-empty 256
    B256 = -(-len(b256) // (2 * _NCORES)) * 2   # round UP to even per core
    b256 += [0] * (B256 * _NCORES - len(b256))  # pad blocks (type 0, empty)
    return b256, b128, B256, B128


def _prepare(x, nodes, fact, params, bias_p, ho_params, ho_bias):
    """Host-side: sort by id, plan blocks, build packed fp16 arrays."""
    N, L = nodes.shape
    E = fact.shape[0]
    R = params.shape[2]
    NP = params.shape[0]           # 169
    MA = int(round(NP ** 0.5))     # 13

    ids = (x[fact[:, 0], 1] * MA + x[fact[:, 0], 2]).astype(np.int64)   # [E]
    perm = np.argsort(ids, kind="stable")
    ids_s = ids[perm]
    fact_s = fact[perm].astype(np.int64)                                 # [E,3]
    counts = np.bincount(ids_s, minlength=NP)                            # [NP]
    off = np.concatenate([[0], np.cumsum(counts)])

    b256, b128, B256, B128 = _plan_blocks(counts)
    NB256, NB128 = len(b256), len(b128)
    P256, P128 = B256 // 2, B128 // 2

    # slot -> sorted-edge-position map (-1 = padding).  256-blocks occupy
    # slots [0, NB256*256); 128-blocks follow.  Per type, its blocks appear
    # in list order (all 256s then its 128), matching edge order.
    tslots = NB256 * 256 + NB128 * 128
    src = np.full(tslots, -1, np.int64)
    used = np.zeros(NP, np.int64)
    cur = {}
    for b, t in enumerate(b256):
        cur.setdefault(t, []).append((b * 256, 256))
    for b, t in enumerate(b128):
        cur.setdefault(t, []).append((NB256 * 256 + b * 128, 128))
    for t, regions in cur.items():
        have = int(counts[t])
        taken = 0
        for start, size in regions:
            n = min(size, have - taken)
            if n <= 0:
                break
            src[start:start + n] = np.arange(off[t] + taken,
                                             off[t] + taken + n)
            taken += n
    valid = src >= 0

    nf = nodes[fact_s].astype(np.float16)                                # [E,3,L]
    featp = np.zeros((tslots, 3, L), np.float16)
    featp[valid] = nf[src[valid]]
    featp = featp[:, [1, 0, 2], :]          # storage order pos = (i1, i0, i2)

    blk_ids = np.array(b256 + b128, np.int64)                # [NB256+NB128]
    Wb = params[blk_ids].astype(np.float16)                  # [NB, L, R]
    HOb = (ho_params[:, blk_ids].astype(np.float16)          # [3, NB, R, L]
           .transpose(1, 2, 0, 3).reshape(-1, R, 3 * L))     # [NB, R, 192]

    NPAIR256, NPAIR128 = NB256 // 2, NB128 // 2
    pkin = np.zeros((max(NPAIR256, 1), 128, 1280), np.float16)
    if NPAIR256:
        pkin[:, :, 0:768] = (
            featp[0:NB256 * 256]
            .reshape(NPAIR256, 2, 2, _SEG, 3, L)   # q c seg e pos l
            .transpose(0, 1, 5, 4, 2, 3)           # q c l pos seg e
            .reshape(NPAIR256, 128, 768))
        pkin[:, :, 768:896] = Wb[0:NB256].reshape(NPAIR256, 2 * L, R)
        pkin[:, :, 896:1280] = (
            HOb[0:NB256].reshape(NPAIR256, 2, R, 192)
            .transpose(0, 2, 1, 3).reshape(NPAIR256, R, 384))
    pkin128 = np.zeros((max(NPAIR128, 1), 128, 896), np.float16)
    if NPAIR128:
        pkin128[:, :, 0:384] = (
            featp[NB256 * 256:]
            .reshape(NPAIR128, 2, _SEG, 3, L)      # q c e pos l
            .transpose(0, 1, 4, 3, 2)              # q c l pos e
            .reshape(NPAIR128, 128, 384))
        pkin128[:, :, 384:512] = Wb[NB256:].reshape(NPAIR128, 2 * L, R)
        pkin128[:, :, 512:896] = (
            HOb[NB256:].reshape(NPAIR128, 2, R, 192)
            .transpose(0, 2, 1, 3).reshape(NPAIR128, R, 384))

    # bias per block, core-major column order: 256-blocks then 128-blocks
    bias_blk = bias_p[blk_ids, 0].astype(np.float32)          # [NB, R]
    biasT = np.zeros((_NCORES, R, B256 + B128), np.float32)
    if NB256:
        biasT[:, :, 0:B256] = (bias_blk[0:NB256]
                               .reshape(_NCORES, B256, R).transpose(0, 2, 1))
    if NB128:
        biasT[:, :, B256:] = (bias_blk[NB256:]
                              .reshape(_NCORES, B128, R).transpose(0, 2, 1))

    return dict(pkin=pkin, pkin128=pkin128, biasT=np.ascontiguousarray(biasT),
                P256=P256, P128=P128, NB256=NB256, NB128=NB128,
                src=src, valid=valid, fact_s=fact_s, ids_s=ids_s,
                N=N, E=E, L=L, tslots=tslots)


def _postprocess(msgs_all, msgs128_all, prep, ho_bias):
    """Decode per-slot messages, add host-side b2, segment-sum into node_msg."""
    N, E, L = prep["N"], prep["E"], prep["L"]
    NB256, NB128 = prep["NB256"], prep["NB128"]
    src, valid, fact_s, ids_s = (prep["src"], prep["valid"],
                                 prep["fact_s"], prep["ids_s"])
    parts = []
    if NB256:
        parts.append(
            msgs_all.astype(np.float32)
            .reshape(NB256 // 2, 2, 64, 3, 2, _SEG)   # q c l i seg e
            .transpose(0, 1, 4, 5, 3, 2)              # q c seg e i l
            .reshape(NB256 * 256, 3, 64))
    if NB128:
        parts.append(
            msgs128_all.astype(np.float32)
            .reshape(NB128 // 2, 2, 64, 3, _SEG)      # q c l i e
            .transpose(0, 1, 4, 3, 2)                 # q c e i l
            .reshape(NB128 * 128, 3, 64))
    slots = np.concatenate(parts, axis=0)

    msg_e = np.empty((E, 3, L), np.float32)
    msg_e[src[valid]] = slots[valid]
    msg_e += ho_bias[:, ids_s, 0].astype(np.float32).transpose(1, 0, 2)  # [E,3,L]

    idx_all = fact_s.T.reshape(-1)                                       # [3E]
    val_all = msg_e.transpose(1, 0, 2).reshape(-1, L)                    # [3E,L]
    order = np.argsort(idx_all, kind="stable")
    idx_sorted = idx_all[order]
    val_sorted = val_all[order]
    uniq, starts = np.unique(idx_sorted, return_index=True)
    sums = np.add.reduceat(val_sorted, starts, axis=0)
    out = np.zeros((N, L), np.float32)
    out[uniq] = sums
    return out


def _run_device(prep, trace=False, trace_kwargs=None):
    from concourse.bass_utils import run_bass_kernel_spmd

    P256, P128 = prep["P256"], prep["P128"]
    nc = _get_program(P256, P128)
    in_maps = []
    for c in range(_NCORES):
        in_maps.append({
            "pkin": prep["pkin"][c * P256:(c + 1) * P256] if P256
            else prep["pkin"],
            "pkin128": prep["pkin128"][c * P128:(c + 1) * P128] if P128
            else prep["pkin128"],
            "bia": prep["biasT"][c],
        })
    kwargs = {}
    if trace:
        kwargs["trace"] = True
        if trace_kwargs:
            kwargs.update(trace_kwargs)
    res = run_bass_kernel_spmd(nc, in_maps, list(range(_NCORES)), **kwargs)
    msgs_all = np.concatenate([res.results[c]["msgs"] for c in range(_NCORES)],
                              axis=0) if P256 else None
    msgs128_all = np.concatenate([res.results[c]["msgs128"]
                                  for c in range(_NCORES)], axis=0) if P128 \
        else None
    return msgs_all, msgs128_all, res


def kernel(x, nodes, fact, fact_dim, params, bias_p, ho_params, ho_bias,
           _trace=False, _trace_kwargs=None):
    x = np.asarray(x)
    nodes = np.asarray(nodes, dtype=np.float32)
    fact = np.asarray(fact)
    params = np.asarray(params)
    bias_p = np.asarray(bias_p)
    ho_params = np.asarray(ho_params)
    ho_bias = np.asarray(ho_bias)

    prep = _prepare(x, nodes, fact, params, bias_p, ho_params, ho_bias)
    msgs_all, msgs128_all, res = _run_device(prep, trace=_trace,
                                             trace_kwargs=_trace_kwargs)
    out = _postprocess(msgs_all, msgs128_all, prep, ho_bias)
    kernel.last_results = res
    return out
